# revision 1
# baseline (speedup 1.0000x reference)
"""Banded (sparse) multi-head attention block on 8 TRN2 NeuronCores.

Reference computation (B=4, N=1024, C=1024, H=16, D=64, epoch=25 -> band w=8):
    qkv = x @ Wqkv.T                      [B,N,3C], per-head interleaved split
    q,k,v per head; score = q k^T / sqrt(D); band mask |i-j|<=8; softmax
    ctx = attn @ v; out = ctx @ Wproj.T + bproj

Sharding: the band mask makes attention local, so we shard the sequence:
core = (b, s) with b in 0..3, s in 0..1 owns tokens [s*512, (s+1)*512) of
batch b plus an 8-token halo on each side.  No collectives are needed.

Per-core pipeline (all layouts chosen so no on-chip transpose of x or the
weights is ever needed; the host pre-transposes instead):
  xt   [C, 528]   = x-slice^T (528 = 8 + 512 + 8, zero-padded at seq ends)
  GEMM-V : v_nat[j, c]   = xt^T-slices @ wvt   (bf16)
  GEMM-QK: qk^T[c', n]   = wqkt^T @ xt         (bf16)
           c' packs head h into partitions: [h][0:64]=q^T (prescaled), [64:128]=k^T
  Attention per (head-pair, 128-row q-block): 160-wide k/v window,
           score -> exp (ACT) -> masked-sum (fused DVE TTR) -> recip ->
           normalize -> PE-transpose -> ctx^T accumulated into [c, n] slabs
  GEMM-O : out^T[o, n] = wpt^T @ ctx^T + bproj (bf16)
"""

import sys

if "/opt/trn_rl_repo" not in sys.path:
    sys.path.insert(0, "/opt/trn_rl_repo")

import numpy as np

B, N, C, H, D = 4, 1024, 1024, 16, 64
NO = 512          # owned tokens per core
HALO = 8
NL = NO + 2 * HALO    # 528 local tokens
KL = 544          # padded k/v length (4*128 + 32)
WW = 160          # score window width per 128-row q block
NBLK = 4          # q blocks of 128 per core
KT = 8            # contraction tiles (1024 / 128)
SCALE = D ** -0.5

_CACHE = {}


def _build_nc():
    import concourse.bacc as bacc
    import concourse.tile as tile
    from concourse import mybir
    from concourse.masks import make_identity
    from contextlib import ExitStack
    from collections import deque

    f32 = mybir.dt.float32
    f32r = mybir.dt.float32r
    bf16 = mybir.dt.bfloat16
    MUL = mybir.AluOpType.mult
    ADD = mybir.AluOpType.add
    EXP = mybir.ActivationFunctionType.Exp

    nc = bacc.Bacc(None, target_bir_lowering=False)

    xt_e = nc.declare_dram_parameter("xt", [C, NL], bf16, isOutput=False)
    wqkb_e = nc.declare_dram_parameter("wqkb", [H, 128, C], bf16, isOutput=False)
    wvt_e = nc.declare_dram_parameter("wvt", [C, C], bf16, isOutput=False)
    wpb_e = nc.declare_dram_parameter("wpb", [8, 128, C], bf16, isOutput=False)
    bp_e = nc.declare_dram_parameter("bp", [128, 8], f32, isOutput=False)
    mask_e = nc.declare_dram_parameter("mask", [128, NBLK * WW], f32, isOutput=False)
    out_e = nc.declare_dram_parameter("outT", [C, NO], f32, isOutput=True)

    with tile.TileContext(nc) as tc, ExitStack() as ctx:
        const = ctx.enter_context(tc.tile_pool(name="const", bufs=1))
        xts = ctx.enter_context(tc.tile_pool(name="xts", bufs=1))
        wv_pool = ctx.enter_context(tc.tile_pool(name="wvp", bufs=1))
        wqk_pool = ctx.enter_context(tc.tile_pool(name="wqkp", bufs=4))
        wp_pool = ctx.enter_context(tc.tile_pool(name="wpp", bufs=3))
        qk_pool = ctx.enter_context(tc.tile_pool(name="qksb", bufs=1))
        v_pool = ctx.enter_context(tc.tile_pool(name="vsb", bufs=1))
        ctx_pool = ctx.enter_context(tc.tile_pool(name="ctxsb", bufs=1))
        att_pool = ctx.enter_context(tc.tile_pool(name="att", bufs=3))
        out_pool = ctx.enter_context(tc.tile_pool(name="outp", bufs=2))

        # ---- constants -------------------------------------------------
        mask_sb = const.tile([128, NBLK * WW], f32)
        nc.sync.dma_start(out=mask_sb[:], in_=mask_e[:])
        bp_sb = const.tile([128, 8], f32)
        nc.sync.dma_start(out=bp_sb[:], in_=bp_e[:])
        ident = const.tile([128, 128], bf16)
        make_identity(nc, ident[:])

        # x^T tiles, resident
        xt_t = []
        for k in range(KT):
            t = xts.tile([128, NL], bf16, tag=f"xt{k}")
            nc.sync.dma_start(out=t[:], in_=xt_e[k * 128:(k + 1) * 128, :])
            xt_t.append(t)

        # wvt resident
        wv_t = []
        for k in range(KT):
            t = wv_pool.tile([128, C], bf16, tag=f"wv{k}")
            nc.sync.dma_start(out=t[:], in_=wvt_e[k * 128:(k + 1) * 128, :])
            wv_t.append(t)

        # persistent activation slabs.  q_sb[hp] = [q_{2hp}|q_{2hp+1}]^T.
        # For k we keep two zero-padded variants so every score matmul can
        # contract over the full 128 partitions at base partition 0 (the
        # hardware exec unit faults on matmuls whose operands sit at base
        # partition 64): kxa[hp] = [k_{2hp} | 0], kxb[hp] = [0 | k_{2hp+1}].
        q_sb, kxa_sb, kxb_sb = [], [], []
        for hp in range(8):
            tq = qk_pool.tile([128, KL], bf16, tag=f"q{hp}")
            q_sb.append(tq)
            ta = qk_pool.tile([128, KL], bf16, tag=f"kxa{hp}")
            nc.vector.memset(ta[:], 0.0)
            kxa_sb.append(ta)
            tb = qk_pool.tile([128, KL], bf16, tag=f"kxb{hp}")
            nc.vector.memset(tb[:], 0.0)
            kxb_sb.append(tb)
        v_sb = []
        for j in range(5):
            t = v_pool.tile([128, C], bf16, tag=f"v{j}")
            v_sb.append(t)
        nc.vector.memset(v_sb[4][:], 0.0)
        ctxT = []
        for cb in range(8):
            t = ctx_pool.tile([128, NO], bf16, tag=f"ctx{cb}")
            ctxT.append(t)

        # ---- phase V: v_nat = x @ Wv^T ---------------------------------
        with tc.tile_pool(name="psv", bufs=2, space="PSUM") as psv:
            for jb in range(5):
                pj = 128 if jb < 4 else 16
                for nch in range(2):
                    ps = psv.tile([128, 512], f32, tag="ps")
                    for k in range(KT):
                        nc.tensor.matmul(
                            ps[:pj, :],
                            lhsT=xt_t[k][:, jb * 128:jb * 128 + pj],
                            rhs=wv_t[k][:, nch * 512:(nch + 1) * 512],
                            start=(k == 0), stop=(k == KT - 1),
                        )
                    nc.any.tensor_copy(
                        out=v_sb[jb][:pj, nch * 512:(nch + 1) * 512], in_=ps[:pj, :])

        # ---- phase QK + attention --------------------------------------
        with tc.tile_pool(name="psqk", bufs=2, space="PSUM") as psqk:

            def emit_qk(g):
                wt = wqk_pool.tile([128, C], bf16, tag="wqk")
                nc.sync.dma_start(out=wt[:], in_=wqkb_e[g])
                for nch, (n0, nn) in enumerate(((0, 264), (264, 264))):
                    ps = psqk.tile([128, 264], f32, tag="psqk")
                    for k in range(KT):
                        nc.tensor.matmul(
                            ps[:],
                            lhsT=wt[:, k * 128:(k + 1) * 128],
                            rhs=xt_t[k][:, n0:n0 + nn],
                            start=(k == 0), stop=(k == KT - 1),
                        )
                    if g % 2 == 0:
                        nc.any.tensor_copy(
                            out=q_sb[g // 2][:, n0:n0 + nn], in_=ps[:])
                    else:
                        nc.any.tensor_copy(
                            out=kxa_sb[g // 2][0:64, n0:n0 + nn], in_=ps[0:64, :])
                        nc.any.tensor_copy(
                            out=kxb_sb[g // 2][64:128, n0:n0 + nn],
                            in_=ps[64:128, :])

            emit_qk(0)
            emit_qk(1)

            with tc.tile_pool(name="pss", bufs=2, space="PSUM") as pss_pool, \
                 tc.tile_pool(name="pst", bufs=2, space="PSUM") as pst_pool, \
                 tc.tile_pool(name="psc", bufs=2, space="PSUM") as psc_pool:
                state = {}

                def emit_score_softmax(it):
                    hp, blk = it
                    j0 = blk * 128
                    q0 = HALO + blk * 128
                    w0 = blk * WW
                    ps = pss_pool.tile([128, 2 * WW], f32, tag="ps_s")
                    for hi, kx in enumerate((kxa_sb, kxb_sb)):
                        nc.tensor.matmul(
                            ps[:, hi * WW:(hi + 1) * WW],
                            lhsT=q_sb[hp][:, q0:q0 + 128],
                            rhs=kx[hp][:, j0:j0 + WW],
                            start=True, stop=True,
                        )
                    ex = att_pool.tile([128, 2 * WW], f32, tag="ex")
                    nc.scalar.activation(out=ex[:], in_=ps[:], func=EXP)
                    tmp = att_pool.tile([128, 2 * WW], f32, tag="tmp")
                    den = att_pool.tile([128, 2], f32, tag="den")
                    for hi in range(2):
                        nc.vector.tensor_mul(
                            out=tmp[:, hi * WW:(hi + 1) * WW],
                            in0=ex[:, hi * WW:(hi + 1) * WW],
                            in1=mask_sb[:, w0:w0 + WW],
                        )
                        nc.vector.reduce_sum(
                            out=den[:, hi:hi + 1],
                            in_=tmp[:, hi * WW:(hi + 1) * WW],
                            axis=mybir.AxisListType.X,
                        )
                    rec = att_pool.tile([128, 2], f32, tag="rec")
                    nc.vector.reciprocal(out=rec[:], in_=den[:])
                    at = att_pool.tile([128, 2 * WW], bf16, tag="at")
                    for hi in range(2):
                        nc.vector.tensor_scalar_mul(
                            out=at[:, hi * WW:(hi + 1) * WW],
                            in0=tmp[:, hi * WW:(hi + 1) * WW],
                            scalar1=rec[:, hi:hi + 1],
                        )
                    state[it] = at

                def emit_transpose_ctx(it):
                    hp, blk = it
                    at = state.pop(it)
                    # pt layout: [:,0:128]=hA main, [:,128:256]=hB main,
                    # [0:32,256:384]=hA tail, [0:32,384:512]=hB tail
                    pt = pst_pool.tile([128, 512], bf16, tag="pt")
                    for hi in range(2):
                        nc.tensor.transpose(
                            pt[:, hi * 128:hi * 128 + 128],
                            at[:, hi * WW:hi * WW + 128], ident[:])
                        nc.tensor.transpose(
                            pt[0:32, 256 + hi * 128:256 + hi * 128 + 128],
                            at[:, hi * WW + 128:hi * WW + 160], ident[:])
                    atT = att_pool.tile([128, 512], bf16, tag="atT")
                    nc.any.tensor_copy(out=atT[:, 0:256], in_=pt[:, 0:256])
                    nc.any.tensor_copy(out=atT[0:32, 256:512], in_=pt[0:32, 256:512])
                    pc = psc_pool.tile([128, 128], f32, tag="pc")
                    for hi in range(2):
                        h = 2 * hp + hi
                        nc.tensor.matmul(
                            pc[hi * 64:(hi + 1) * 64, :],
                            lhsT=v_sb[blk][:, h * 64:(h + 1) * 64],
                            rhs=atT[:, hi * 128:hi * 128 + 128],
                            start=True, stop=False,
                        )
                        nc.tensor.matmul(
                            pc[hi * 64:(hi + 1) * 64, :],
                            lhsT=v_sb[blk + 1][0:32, h * 64:(h + 1) * 64],
                            rhs=atT[0:32, 256 + hi * 128:256 + hi * 128 + 128],
                            start=False, stop=True,
                        )
                    nc.any.tensor_copy(
                        out=ctxT[hp][:, blk * 128:(blk + 1) * 128], in_=pc[:])

                pending = deque()
                for hp in range(8):
                    for g in (2 * hp + 2, 2 * hp + 3):
                        if g < H:
                            emit_qk(g)
                    for blk in range(NBLK):
                        emit_score_softmax((hp, blk))
                        pending.append((hp, blk))
                        if len(pending) > 2:
                            emit_transpose_ctx(pending.popleft())
                while pending:
                    emit_transpose_ctx(pending.popleft())

        # ---- phase O: out^T = Wproj^T-contraction over ctx^T + bias ----
        with tc.tile_pool(name="psg2", bufs=2, space="PSUM") as psg2:
            for ob in range(8):
                wt = wp_pool.tile([128, C], bf16, tag="wp")
                nc.sync.dma_start(out=wt[:], in_=wpb_e[ob])
                ps = psg2.tile([128, NO], f32, tag="po")
                for k in range(KT):
                    nc.tensor.matmul(
                        ps[:],
                        lhsT=wt[:, k * 128:(k + 1) * 128],
                        rhs=ctxT[k][:],
                        start=(k == 0), stop=(k == KT - 1),
                    )
                ot = out_pool.tile([128, NO], f32, tag="ot")
                nc.vector.tensor_scalar_add(
                    out=ot[:], in0=ps[:], scalar1=bp_sb[:, ob:ob + 1])
                nc.sync.dma_start(
                    out=out_e[ob * 128:(ob + 1) * 128, :], in_=ot[:])

    nc.compile()
    return nc


def _get_nc():
    if "nc" not in _CACHE:
        _CACHE["nc"] = _build_nc()
    return _CACHE["nc"]


def _band_width(epoch):
    if epoch is None or epoch >= 50:
        return None
    if epoch < 20:
        return 6
    if epoch < 30:
        return 8
    if epoch < 40:
        return 10
    return 12


def _numpy_ref(x, Wqkv, Wproj, bproj, w):
    """Pure-numpy fallback for band widths this kernel wasn't compiled for."""
    b, n, c = x.shape
    d = c // H
    qkv = np.einsum("bnc,oc->bno", x, Wqkv)
    qkv = qkv.reshape(b, n, H, 3 * d).transpose(0, 2, 1, 3)
    q, k, v = np.split(qkv, 3, axis=-1)
    score = np.einsum("bhid,bhjd->bhij", q, k) * (d ** -0.5)
    if w is not None:
        idx = np.arange(n)
        mask = np.abs(idx[:, None] - idx[None, :]) <= w
        score = np.where(mask[None, None], score, np.float32(-1e9))
    score -= score.max(axis=-1, keepdims=True)
    e = np.exp(score)
    attn = e / e.sum(axis=-1, keepdims=True)
    ctxv = np.einsum("bhij,bhjd->bhid", attn, v)
    ctxv = ctxv.transpose(0, 2, 1, 3).reshape(b, n, c)
    return (np.einsum("bnc,oc->bno", ctxv, Wproj) + bproj).astype(np.float32)


def _prep_in_maps(x, Wqkv, Wproj, bproj):
    import ml_dtypes
    bf = ml_dtypes.bfloat16
    x = np.ascontiguousarray(np.asarray(x, dtype=np.float32))
    Wqkv = np.asarray(Wqkv, dtype=np.float32)
    Wproj = np.asarray(Wproj, dtype=np.float32)
    bproj = np.asarray(bproj, dtype=np.float32)

    # qk weight output-blocks g: even g -> [q_{2hp} | q_{2hp+1}] (prescaled),
    # odd g -> [k_{2hp} | k_{2hp+1}]
    wsplit = Wqkv.reshape(H, 3, D, C)
    wq = wsplit[:, 0] * np.float32(SCALE)                      # [H, D, C]
    wk = wsplit[:, 1]                                          # [H, D, C]
    wv = wsplit[:, 2]                                          # [H, D, C]
    wg = np.empty((H, 128, C), dtype=np.float32)
    wg[0::2] = wq.reshape(8, 128, C)
    wg[1::2] = wk.reshape(8, 128, C)
    # wqkb[g, p, k*128+m] = wg[g, m, k*128+p]: per-g contiguous [128, C]
    # slabs whose col-block k is the k-th contraction tile's lhsT
    wqkb = np.ascontiguousarray(
        wg.transpose(0, 2, 1).reshape(H, KT, 128, 128).transpose(0, 2, 1, 3)
        .reshape(H, 128, C)).astype(bf)
    wvt = np.ascontiguousarray(wv.reshape(H * D, C).T).astype(bf)  # [C, C]
    wpb = np.ascontiguousarray(                                 # Wproj^T blocked
        Wproj.T.reshape(KT, 128, 8, 128).transpose(2, 1, 0, 3)
        .reshape(8, 128, C)).astype(bf)
    bp = np.ascontiguousarray(bproj.reshape(8, 128).T)         # [128, 8]

    # masks per sequence-half s: 1.0 where in-band and the k column is a
    # real token, else 0.0
    r = np.arange(128)[:, None]
    jj = np.arange(WW)[None, :]
    band = (jj >= r) & (jj <= r + 2 * HALO)                    # [128, WW]
    masks = []
    for s in (0, 1):
        m = np.zeros((128, NBLK * WW), dtype=np.float32)
        for blk in range(NBLK):
            mloc = blk * 128 + jj                              # local k index
            valid = (mloc >= HALO) if s == 0 else (mloc < NO + HALO)
            m[:, blk * WW:(blk + 1) * WW] = (band & valid).astype(np.float32)
        masks.append(m)

    in_maps = []
    for core in range(8):
        b, s = core // 2, core % 2
        xloc = np.zeros((NL, C), dtype=np.float32)
        g0 = s * NO - HALO
        lo, hi = max(0, g0), min(N, g0 + NL)
        xloc[lo - g0:hi - g0] = x[b, lo:hi]
        in_maps.append({
            "xt": np.ascontiguousarray(xloc.T).astype(bf),
            "wqkb": wqkb, "wvt": wvt, "wpb": wpb, "bp": bp,
            "mask": masks[s],
        })
    return in_maps


def kernel(x, Wqkv, Wproj, bproj, epoch):
    ep = None if epoch is None else int(np.asarray(epoch))
    w = _band_width(ep)
    if w != HALO:
        return _numpy_ref(np.asarray(x, np.float32), np.asarray(Wqkv, np.float32),
                          np.asarray(Wproj, np.float32),
                          np.asarray(bproj, np.float32), w)

    from concourse.bass_utils import run_bass_kernel_spmd

    nc = _get_nc()
    in_maps = _prep_in_maps(x, Wqkv, Wproj, bproj)
    res = run_bass_kernel_spmd(nc, in_maps, core_ids=list(range(8)))
    _CACHE["last_results"] = res

    out = np.empty((B, N, C), dtype=np.float32)
    for core in range(8):
        b, s = core // 2, core % 2
        out[b, s * NO:(s + 1) * NO, :] = res.results[core]["outT"].T
    return out



# revision 8
# speedup vs baseline: 1.3311x; 1.3311x over previous
"""Banded (sparse) multi-head attention block on 8 TRN2 NeuronCores.

Reference computation (B=4, N=1024, C=1024, H=16, D=64, epoch=25 -> band w=8):
    qkv = x @ Wqkv.T                      [B,N,3C], per-head interleaved split
    q,k,v per head; score = q k^T / sqrt(D); band mask |i-j|<=8; softmax
    ctx = attn @ v; out = ctx @ Wproj.T + bproj

Sharding: the band mask makes attention local, so we shard the sequence:
core = (b, s) with b in 0..3, s in 0..1 owns tokens [s*512, (s+1)*512) of
batch b plus an 8-token halo on each side.  No collectives are needed.

Per-core pipeline (tuned for the tensor engine):
  GEMM-V : v_nat[j, c]  with xt-stationary, dual-PSUM k-inner accumulation
  GEMM-QK: qk^T[c', n]  with w-stationary, dual-PSUM k-inner accumulation
  Attention per (head-pair, 128-row q-block, 144-wide window):
      band mask preloaded into PSUM as additive -1e9 bias (identity matmul),
      scores accumulate on top, exp on ACT with accum_out row-sums (=denoms),
      reciprocal+normalize on DVE, PE-transpose, ctx accumulated per-hp into
      one PSUM bank -> ctxT[hp] slabs [c, n]
  GEMM-O : out_nat[n, o] with ctxT-stationary, dual-PSUM k-inner, bias via
      DVE add during PSUM->SBUF eviction
A post-schedule pass removes back-to-back redundant LDWEIGHTS so paired
matmuls share one stationary load.
"""

import sys

if "/opt/trn_rl_repo" not in sys.path:
    sys.path.insert(0, "/opt/trn_rl_repo")

import numpy as np

B, N, C, H, D = 4, 1024, 1024, 16, 64
NO = 512          # owned tokens per core
HALO = 8
NL = NO + 2 * HALO    # 528 local tokens
WW = 144          # score window width per 128-row q block (128 main + 16 tail)
NBLK = 4          # q blocks of 128 per core
KT = 8            # contraction tiles (1024 / 128)
SCALE = D ** -0.5
NEG = -1.0e9

_CACHE = {}


def _dedup_ldweights(nc, mybir):
    """Remove InstLdweights whose stationary AP + flags match the previous
    ldweights on the PE stream with no intervening control flow.  Sync info
    on a removed duplicate is transferred to the next instruction."""
    removed = 0
    for fn in nc.m.functions:
        for blk in fn.blocks:
            insts = blk.instructions
            last_key = None
            drops = []
            for idx, inst in enumerate(insts):
                tname = type(inst).__name__
                if isinstance(inst, mybir.InstLdweights):
                    key = (
                        repr(inst.ins[0]),
                        getattr(inst, "is_transpose", None),
                        getattr(inst, "perf_mode", None),
                        getattr(inst, "tile_position", None),
                        getattr(inst, "tile_size", None),
                    )
                    if key == last_key:
                        drops.append((idx, inst))
                    else:
                        last_key = key
                elif isinstance(inst, mybir.InstMatmult):
                    pass  # does not change the loaded stationary
                elif "Branch" in tname or "ControlFlow" in tname or "Call" in tname:
                    last_key = None
            for idx, inst in drops:
                if inst.has_wait() or inst.has_update():
                    nxt = insts[idx + 1] if idx + 1 < len(insts) else None
                    if nxt is None:
                        continue
                    nxt.add_sync_dependencies_from(inst)
                insts.remove(inst)
                removed += 1
    return removed


def _build_nc():
    import concourse.bacc as bacc
    import concourse.tile as tile
    from concourse import mybir
    from concourse.masks import make_identity
    from contextlib import ExitStack

    f32 = mybir.dt.float32
    bf16 = mybir.dt.bfloat16
    EXP = mybir.ActivationFunctionType.Exp

    nc = bacc.Bacc(None, target_bir_lowering=False)

    xt_e = nc.declare_dram_parameter("xt", [C, NL], bf16, isOutput=False)
    wqkb_e = nc.declare_dram_parameter("wqkb", [H, 128, C], bf16, isOutput=False)
    wvt_e = nc.declare_dram_parameter("wvt", [C, C], bf16, isOutput=False)
    wpt_e = nc.declare_dram_parameter("wpt", [KT, 128, C], bf16, isOutput=False)
    bpb_e = nc.declare_dram_parameter("bpb", [128, C], bf16, isOutput=False)
    bias_e = nc.declare_dram_parameter("bias", [128, NBLK * 2 * WW], bf16,
                                       isOutput=False)
    out_e = nc.declare_dram_parameter("out", [NO, C], f32, isOutput=True)

    with tile.TileContext(nc) as tc, ExitStack() as ctx:
        const = ctx.enter_context(tc.tile_pool(name="const", bufs=1))
        xts = ctx.enter_context(tc.tile_pool(name="xts", bufs=1))
        wv_pool = ctx.enter_context(tc.tile_pool(name="wvp", bufs=1))
        wqk_pool = ctx.enter_context(tc.tile_pool(name="wqkp", bufs=1))
        wpt_pool = ctx.enter_context(tc.tile_pool(name="wptp", bufs=1))
        qk_pool = ctx.enter_context(tc.tile_pool(name="qksb", bufs=1))
        v_pool = ctx.enter_context(tc.tile_pool(name="vsb", bufs=1))
        ctx_pool = ctx.enter_context(tc.tile_pool(name="ctxsb", bufs=1))
        ex_pool = ctx.enter_context(tc.tile_pool(name="exp", bufs=4))
        at_pool = ctx.enter_context(tc.tile_pool(name="atp", bufs=8))
        atT_pool = ctx.enter_context(tc.tile_pool(name="atTp", bufs=2))
        dn_pool = ctx.enter_context(tc.tile_pool(name="dnp", bufs=4))
        out_pool = ctx.enter_context(tc.tile_pool(name="outp", bufs=3))

        # ---- DMAs (queue order == program order: feed V phase first) ----
        xt_t = []
        for k in range(KT):
            t = xts.tile([128, NL], bf16, tag=f"xt{k}")
            nc.sync.dma_start(out=t[:], in_=xt_e[k * 128:(k + 1) * 128, :])
            xt_t.append(t)
        wv_t = []
        for k in range(KT):
            t = wv_pool.tile([128, C], bf16, tag=f"wv{k}")
            nc.sync.dma_start(out=t[:], in_=wvt_e[k * 128:(k + 1) * 128, :])
            wv_t.append(t)
        bias_sb = const.tile([128, NBLK * 2 * WW], bf16, tag="bias")
        nc.sync.dma_start(out=bias_sb[:], in_=bias_e[:])
        wqk_t = []
        for g in range(H):
            t = wqk_pool.tile([128, C], bf16, tag=f"wqk{g}")
            nc.sync.dma_start(out=t[:], in_=wqkb_e[g])
            wqk_t.append(t)
        wpt_t = []
        for k in range(KT):
            t = wpt_pool.tile([128, C], bf16, tag=f"wpt{k}")
            nc.sync.dma_start(out=t[:], in_=wpt_e[k])
            wpt_t.append(t)
        bpb_sb = const.tile([128, C], bf16, tag="bpb")
        nc.sync.dma_start(out=bpb_sb[:], in_=bpb_e[:])

        ident = const.tile([128, 128], bf16, tag="ident")
        make_identity(nc, ident[:])

        # persistent activation slabs; kxa = [k_even | 0], kxb = [0 | k_odd]
        # so score matmuls contract the full 128 partitions at base 0.
        q_sb, kxa_sb, kxb_sb = [], [], []
        for hp in range(8):
            tq = qk_pool.tile([128, NL], bf16, tag=f"q{hp}")
            q_sb.append(tq)
            ta = qk_pool.tile([128, NL], bf16, tag=f"kxa{hp}")
            nc.vector.memset(ta[64:128, :], 0.0)
            kxa_sb.append(ta)
            tb = qk_pool.tile([128, NL], bf16, tag=f"kxb{hp}")
            nc.vector.memset(tb[0:64, :], 0.0)
            kxb_sb.append(tb)
        v_sb = []
        for j in range(5):
            t = v_pool.tile([128, C], bf16, tag=f"v{j}")
            v_sb.append(t)
        ctxT = []
        for cb in range(8):
            t = ctx_pool.tile([128, NO], bf16, tag=f"ctx{cb}")
            ctxT.append(t)

        # ---- phase V: v_nat = x @ Wv^T (xt-stationary, k-inner) -------
        with tc.tile_pool(name="psv", bufs=2, space="PSUM") as psv:
            for jb in range(5):
                pj = 128 if jb < 4 else 16
                ps0 = psv.tile([128, 512], f32, tag="psv0")
                ps1 = psv.tile([128, 512], f32, tag="psv1")
                for k in range(KT):
                    nc.tensor.matmul(
                        ps0[:pj, :],
                        lhsT=xt_t[k][:, jb * 128:jb * 128 + pj],
                        rhs=wv_t[k][:, 0:512],
                        start=(k == 0), stop=(k == KT - 1),
                    )
                    nc.tensor.matmul(
                        ps1[:pj, :],
                        lhsT=xt_t[k][:, jb * 128:jb * 128 + pj],
                        rhs=wv_t[k][:, 512:1024],
                        start=(k == 0), stop=(k == KT - 1),
                    )
                nc.scalar.copy(out=v_sb[jb][:pj, 0:512], in_=ps0[:pj, :])
                nc.scalar.copy(out=v_sb[jb][:pj, 512:1024], in_=ps1[:pj, :])

        # ---- main: QK GEMM interleaved with attention ------------------
        with tc.tile_pool(name="psqk", bufs=2, space="PSUM") as psqk, \
             tc.tile_pool(name="pss", bufs=2, space="PSUM") as pss_pool, \
             tc.tile_pool(name="pst", bufs=1, space="PSUM") as pst_pool, \
             tc.tile_pool(name="psc", bufs=1, space="PSUM") as psc_pool:
            state = {}

            def emit_qk(g):
                wt = wqk_t[g]
                ps0 = psqk.tile([128, 264], f32, tag="psqk0")
                ps1 = psqk.tile([128, 264], f32, tag="psqk1")
                for k in range(KT):
                    nc.tensor.matmul(
                        ps0[:],
                        lhsT=wt[:, k * 128:(k + 1) * 128],
                        rhs=xt_t[k][:, 0:264],
                        start=(k == 0), stop=(k == KT - 1),
                    )
                    nc.tensor.matmul(
                        ps1[:],
                        lhsT=wt[:, k * 128:(k + 1) * 128],
                        rhs=xt_t[k][:, 264:528],
                        start=(k == 0), stop=(k == KT - 1),
                    )
                if g % 2 == 0:
                    nc.scalar.copy(out=q_sb[g // 2][:, 0:264], in_=ps0[:])
                    nc.vector.tensor_copy(out=q_sb[g // 2][:, 264:528], in_=ps1[:])
                else:
                    nc.scalar.copy(
                        out=kxa_sb[g // 2][0:64, 0:264], in_=ps0[0:64, :])
                    nc.vector.tensor_copy(
                        out=kxb_sb[g // 2][64:128, 0:264], in_=ps0[64:128, :])
                    nc.scalar.copy(
                        out=kxa_sb[g // 2][0:64, 264:528], in_=ps1[0:64, :])
                    nc.vector.tensor_copy(
                        out=kxb_sb[g // 2][64:128, 264:528], in_=ps1[64:128, :])

            def emit_scores(hp, blk):
                j0 = blk * 128
                q0 = HALO + blk * 128
                b0 = blk * 2 * WW
                ps = pss_pool.tile([128, 2 * WW], f32, tag="pss")
                nc.tensor.matmul(
                    ps[:, 0:WW], lhsT=ident[:, 0:128],
                    rhs=bias_sb[:, b0:b0 + WW], start=True, stop=False)
                nc.tensor.matmul(
                    ps[:, 0:WW], lhsT=q_sb[hp][:, q0:q0 + 128],
                    rhs=kxa_sb[hp][:, j0:j0 + WW], start=False, stop=True)
                nc.tensor.matmul(
                    ps[:, WW:2 * WW], lhsT=ident[:, 0:128],
                    rhs=bias_sb[:, b0 + WW:b0 + 2 * WW], start=True, stop=False)
                nc.tensor.matmul(
                    ps[:, WW:2 * WW], lhsT=q_sb[hp][:, q0:q0 + 128],
                    rhs=kxb_sb[hp][:, j0:j0 + WW], start=False, stop=True)
                ex = ex_pool.tile([128, 2 * WW], bf16, tag="ex")
                den = dn_pool.tile([128, 4], f32, tag="den")
                for hi in range(2):
                    nc.scalar.activation(
                        out=ex[:, hi * WW:(hi + 1) * WW],
                        in_=ps[:, hi * WW:(hi + 1) * WW],
                        func=EXP, accum_out=den[:, hi:hi + 1])
                nc.vector.reciprocal(out=den[:, 2:4], in_=den[:, 0:2])
                at = at_pool.tile([128, 2 * WW], bf16, tag="at")
                for hi in range(2):
                    nc.vector.tensor_scalar_mul(
                        out=at[:, hi * WW:(hi + 1) * WW],
                        in0=ex[:, hi * WW:(hi + 1) * WW],
                        scalar1=den[:, 2 + hi:3 + hi])
                state[(hp, blk)] = at

            def emit_attnout(hp):
                pc = psc_pool.tile([128, NO], f32, tag="psc")
                for blk in range(NBLK):
                    at = state.pop((hp, blk))
                    # pt: [0:128]=hA main^T, [128:256]=hB main^T,
                    #     [0:16, 256:384]=hA tail^T, [0:16, 384:512]=hB tail^T
                    pt = pst_pool.tile([128, 512], bf16, tag="pt")
                    nc.tensor.transpose(pt[:, 0:128], at[:, 0:128], ident[:])
                    nc.tensor.transpose(pt[:, 128:256], at[:, WW:WW + 128],
                                        ident[:])
                    nc.tensor.transpose(pt[0:16, 256:384], at[:, 128:WW],
                                        ident[:])
                    nc.tensor.transpose(pt[0:16, 384:512], at[:, WW + 128:2 * WW],
                                        ident[:])
                    atT = atT_pool.tile([128, 512], bf16, tag="atT")
                    nc.vector.tensor_copy(out=atT[:, 0:256], in_=pt[:, 0:256])
                    nc.vector.tensor_copy(out=atT[0:16, 256:512],
                                          in_=pt[0:16, 256:512])
                    for hi in range(2):
                        h = 2 * hp + hi
                        nc.tensor.matmul(
                            pc[hi * 64:(hi + 1) * 64, blk * 128:(blk + 1) * 128],
                            lhsT=v_sb[blk][:, h * 64:(h + 1) * 64],
                            rhs=atT[:, hi * 128:hi * 128 + 128],
                            start=True, stop=False,
                        )
                        nc.tensor.matmul(
                            pc[hi * 64:(hi + 1) * 64, blk * 128:(blk + 1) * 128],
                            lhsT=v_sb[blk + 1][0:16, h * 64:(h + 1) * 64],
                            rhs=atT[0:16, 256 + hi * 128:256 + hi * 128 + 128],
                            start=False, stop=True,
                        )
                nc.scalar.copy(out=ctxT[hp][:], in_=pc[:])

            emit_qk(0)
            emit_qk(1)
            for hp in range(8):
                emit_scores(hp, 0)
                emit_scores(hp, 1)
                if 2 * hp + 2 < H:
                    emit_qk(2 * hp + 2)
                emit_scores(hp, 2)
                emit_scores(hp, 3)
                if 2 * hp + 3 < H:
                    emit_qk(2 * hp + 3)
                if hp >= 1:
                    emit_attnout(hp - 1)
            emit_attnout(7)

        # ---- phase O: out_nat[n, o] (ctxT-stationary, k-inner) ---------
        with tc.tile_pool(name="pso", bufs=2, space="PSUM") as pso:
            for t in range(4):
                ps0 = pso.tile([128, 512], f32, tag="pso0")
                ps1 = pso.tile([128, 512], f32, tag="pso1")
                for k in range(KT):
                    nc.tensor.matmul(
                        ps0[:],
                        lhsT=ctxT[k][:, t * 128:(t + 1) * 128],
                        rhs=wpt_t[k][:, 0:512],
                        start=(k == 0), stop=(k == KT - 1),
                    )
                    nc.tensor.matmul(
                        ps1[:],
                        lhsT=ctxT[k][:, t * 128:(t + 1) * 128],
                        rhs=wpt_t[k][:, 512:1024],
                        start=(k == 0), stop=(k == KT - 1),
                    )
                for oh, psx in ((0, ps0), (1, ps1)):
                    ot = out_pool.tile([128, 512], f32, tag="ot")
                    nc.vector.tensor_add(
                        out=ot[:], in0=psx[:],
                        in1=bpb_sb[:, oh * 512:(oh + 1) * 512])
                    nc.sync.dma_start(
                        out=out_e[t * 128:(t + 1) * 128, oh * 512:(oh + 1) * 512],
                        in_=ot[:])

    from concourse import mybir as _mb
    import os as _os
    if _os.environ.get("KERNEL_NO_DEDUP"):
        n = 0
    else:
        n = _dedup_ldweights(nc, _mb)
    _CACHE["ldweights_removed"] = n
    nc.compile()
    return nc


def _get_nc():
    if "nc" not in _CACHE:
        _CACHE["nc"] = _build_nc()
    return _CACHE["nc"]


def _band_width(epoch):
    if epoch is None or epoch >= 50:
        return None
    if epoch < 20:
        return 6
    if epoch < 30:
        return 8
    if epoch < 40:
        return 10
    return 12


def _numpy_ref(x, Wqkv, Wproj, bproj, w):
    """Pure-numpy fallback for band widths this kernel wasn't compiled for."""
    b, n, c = x.shape
    d = c // H
    qkv = np.einsum("bnc,oc->bno", x, Wqkv)
    qkv = qkv.reshape(b, n, H, 3 * d).transpose(0, 2, 1, 3)
    q, k, v = np.split(qkv, 3, axis=-1)
    score = np.einsum("bhid,bhjd->bhij", q, k) * (d ** -0.5)
    if w is not None:
        idx = np.arange(n)
        mask = np.abs(idx[:, None] - idx[None, :]) <= w
        score = np.where(mask[None, None], score, np.float32(-1e9))
    score -= score.max(axis=-1, keepdims=True)
    e = np.exp(score)
    attn = e / e.sum(axis=-1, keepdims=True)
    ctxv = np.einsum("bhij,bhjd->bhid", attn, v)
    ctxv = ctxv.transpose(0, 2, 1, 3).reshape(b, n, c)
    return (np.einsum("bnc,oc->bno", ctxv, Wproj) + bproj).astype(np.float32)


def _prep_in_maps(x, Wqkv, Wproj, bproj):
    import ml_dtypes
    bf = ml_dtypes.bfloat16
    x = np.ascontiguousarray(np.asarray(x, dtype=np.float32))
    Wqkv = np.asarray(Wqkv, dtype=np.float32)
    Wproj = np.asarray(Wproj, dtype=np.float32)
    bproj = np.asarray(bproj, dtype=np.float32)

    # qk weight output-blocks g: even g -> [q_{2hp} | q_{2hp+1}] (prescaled),
    # odd g -> [k_{2hp} | k_{2hp+1}]
    wsplit = Wqkv.reshape(H, 3, D, C)
    wq = wsplit[:, 0] * np.float32(SCALE)                      # [H, D, C]
    wk = wsplit[:, 1]                                          # [H, D, C]
    wv = wsplit[:, 2]                                          # [H, D, C]
    wg = np.empty((H, 128, C), dtype=np.float32)
    wg[0::2] = wq.reshape(8, 128, C)
    wg[1::2] = wk.reshape(8, 128, C)
    # wqkb[g, p, k*128+m] = wg[g, m, k*128+p]: per-g contiguous [128, C]
    # slabs whose col-block k is the k-th contraction tile's lhsT
    wqkb = np.ascontiguousarray(
        wg.transpose(0, 2, 1).reshape(H, KT, 128, 128).transpose(0, 2, 1, 3)
        .reshape(H, 128, C)).astype(bf)
    wvt = np.ascontiguousarray(wv.reshape(H * D, C).T).astype(bf)  # [C, C]
    wpt = np.ascontiguousarray(Wproj.T.reshape(KT, 128, C)).astype(bf)
    bpb = np.ascontiguousarray(
        np.broadcast_to(bproj[None, :], (128, C))).astype(bf)

    # additive score bias per sequence-half s: 0 where in-band and the k
    # column is a real token, else -1e9.  Layout [128, blk*(2*WW)] with the
    # per-block [128, WW] pattern duplicated for the two heads of a pair.
    r = np.arange(128)[:, None]
    jj = np.arange(WW)[None, :]
    band = (jj >= r) & (jj <= r + 2 * HALO)                    # [128, WW]
    biases = []
    for s in (0, 1):
        m = np.full((128, NBLK * 2 * WW), NEG, dtype=np.float32)
        for blk in range(NBLK):
            mloc = blk * 128 + jj                              # local k index
            valid = (mloc >= HALO) if s == 0 else (mloc < NO + HALO)
            bb = np.where(band & valid, 0.0, NEG).astype(np.float32)
            m[:, blk * 2 * WW:blk * 2 * WW + WW] = bb
            m[:, blk * 2 * WW + WW:(blk + 1) * 2 * WW] = bb
        biases.append(m.astype(bf))

    in_maps = []
    for core in range(8):
        b, s = core // 2, core % 2
        xloc = np.zeros((NL, C), dtype=np.float32)
        g0 = s * NO - HALO
        lo, hi = max(0, g0), min(N, g0 + NL)
        xloc[lo - g0:hi - g0] = x[b, lo:hi]
        in_maps.append({
            "xt": np.ascontiguousarray(xloc.T).astype(bf),
            "wqkb": wqkb, "wvt": wvt, "wpt": wpt, "bpb": bpb,
            "bias": biases[s],
        })
    return in_maps


def kernel(x, Wqkv, Wproj, bproj, epoch):
    ep = None if epoch is None else int(np.asarray(epoch))
    w = _band_width(ep)
    if w != HALO:
        return _numpy_ref(np.asarray(x, np.float32), np.asarray(Wqkv, np.float32),
                          np.asarray(Wproj, np.float32),
                          np.asarray(bproj, np.float32), w)

    from concourse.bass_utils import run_bass_kernel_spmd

    nc = _get_nc()
    in_maps = _prep_in_maps(x, Wqkv, Wproj, bproj)
    res = run_bass_kernel_spmd(nc, in_maps, core_ids=list(range(8)))
    _CACHE["last_results"] = res

    out = np.empty((B, N, C), dtype=np.float32)
    for core in range(8):
        b, s = core // 2, core % 2
        out[b, s * NO:(s + 1) * NO, :] = res.results[core]["out"]
    return out


# revision 10
# speedup vs baseline: 1.3646x; 1.0252x over previous
"""Banded (sparse) multi-head attention block on 8 TRN2 NeuronCores.

Reference computation (B=4, N=1024, C=1024, H=16, D=64, epoch=25 -> band w=8):
    qkv = x @ Wqkv.T                      [B,N,3C], per-head interleaved split
    q,k,v per head; score = q k^T / sqrt(D); band mask |i-j|<=8; softmax
    ctx = attn @ v; out = ctx @ Wproj.T + bproj

Sharding: the band mask makes attention local, so we shard the sequence:
core = (b, s) with b in 0..3, s in 0..1 owns tokens [s*512, (s+1)*512) of
batch b plus an 8-token halo on each side.  No collectives are needed.

Per-core pipeline (tuned for the tensor engine):
  GEMM-V : v_nat[j, c]  with xt-stationary, dual-PSUM k-inner accumulation
  GEMM-QK: qk^T[c', n]  with w-stationary, dual-PSUM k-inner accumulation
  Attention per (head-pair, 128-row q-block, 144-wide window):
      band mask preloaded into PSUM as additive -1e9 bias (identity matmul),
      scores accumulate on top, exp on ACT with accum_out row-sums (=denoms),
      reciprocal+normalize on DVE, PE-transpose, ctx accumulated per-hp into
      one PSUM bank -> ctxT[hp] slabs [c, n]
  GEMM-O : out_nat[n, o] with ctxT-stationary, dual-PSUM k-inner, bias via
      DVE add during PSUM->SBUF eviction
A post-schedule pass removes back-to-back redundant LDWEIGHTS so paired
matmuls share one stationary load.
"""

import sys

if "/opt/trn_rl_repo" not in sys.path:
    sys.path.insert(0, "/opt/trn_rl_repo")

import numpy as np

B, N, C, H, D = 4, 1024, 1024, 16, 64
NO = 512          # owned tokens per core
HALO = 8
NL = NO + 2 * HALO    # 528 local tokens
WW = 144          # score window width per 128-row q block (128 main + 16 tail)
NBLK = 4          # q blocks of 128 per core
KT = 8            # contraction tiles (1024 / 128)
SCALE = D ** -0.5
NEG = -1.0e9

_CACHE = {}


def _dedup_ldweights(nc, mybir):
    """Remove InstLdweights whose stationary AP + flags match the previous
    ldweights on the PE stream with no intervening control flow.  Sync info
    on a removed duplicate is transferred to the next instruction."""
    removed = 0
    for fn in nc.m.functions:
        for blk in fn.blocks:
            insts = blk.instructions
            last_key = None
            drops = []
            for idx, inst in enumerate(insts):
                tname = type(inst).__name__
                if isinstance(inst, mybir.InstLdweights):
                    key = (
                        repr(inst.ins[0]),
                        getattr(inst, "is_transpose", None),
                        getattr(inst, "perf_mode", None),
                        getattr(inst, "tile_position", None),
                        getattr(inst, "tile_size", None),
                    )
                    if key == last_key:
                        drops.append((idx, inst))
                    else:
                        last_key = key
                elif isinstance(inst, mybir.InstMatmult):
                    pass  # does not change the loaded stationary
                elif "Branch" in tname or "ControlFlow" in tname or "Call" in tname:
                    last_key = None
            for idx, inst in drops:
                if inst.has_wait() or inst.has_update():
                    nxt = insts[idx + 1] if idx + 1 < len(insts) else None
                    if nxt is None:
                        continue
                    nxt.add_sync_dependencies_from(inst)
                insts.remove(inst)
                removed += 1
    return removed


def _build_nc():
    import concourse.bacc as bacc
    import concourse.tile as tile
    from concourse import mybir
    from concourse.masks import make_identity
    from contextlib import ExitStack

    f32 = mybir.dt.float32
    bf16 = mybir.dt.bfloat16
    EXP = mybir.ActivationFunctionType.Exp

    nc = bacc.Bacc(None, target_bir_lowering=False)

    xt_e = nc.declare_dram_parameter("xt", [C, NL], bf16, isOutput=False)
    wqkb_e = nc.declare_dram_parameter("wqkb", [H, 128, C], bf16, isOutput=False)
    wvt_e = nc.declare_dram_parameter("wvt", [C, C], bf16, isOutput=False)
    wpt_e = nc.declare_dram_parameter("wpt", [KT, 128, C], bf16, isOutput=False)
    bpb_e = nc.declare_dram_parameter("bpb", [128, C], bf16, isOutput=False)
    bias_e = nc.declare_dram_parameter("bias", [128, NBLK * 2 * WW], bf16,
                                       isOutput=False)
    out_e = nc.declare_dram_parameter("out", [NO, C], f32, isOutput=True)

    with tile.TileContext(nc) as tc, ExitStack() as ctx:
        const = ctx.enter_context(tc.tile_pool(name="const", bufs=1))
        xts = ctx.enter_context(tc.tile_pool(name="xts", bufs=1))
        wv_pool = ctx.enter_context(tc.tile_pool(name="wvp", bufs=1))
        wqk_pool = ctx.enter_context(tc.tile_pool(name="wqkp", bufs=1))
        wpt_pool = ctx.enter_context(tc.tile_pool(name="wptp", bufs=1))
        qk_pool = ctx.enter_context(tc.tile_pool(name="qksb", bufs=1))
        v_pool = ctx.enter_context(tc.tile_pool(name="vsb", bufs=1))
        ctx_pool = ctx.enter_context(tc.tile_pool(name="ctxsb", bufs=1))
        ex_pool = ctx.enter_context(tc.tile_pool(name="exp", bufs=4))
        at_pool = ctx.enter_context(tc.tile_pool(name="atp", bufs=8))
        atT_pool = ctx.enter_context(tc.tile_pool(name="atTp", bufs=2))
        dn_pool = ctx.enter_context(tc.tile_pool(name="dnp", bufs=4))
        out_pool = ctx.enter_context(tc.tile_pool(name="outp", bufs=3))

        # ---- DMAs (queue order == program order: qk0/qk1 feed first) ----
        wqk_t = []
        for g in range(H):
            t = wqk_pool.tile([128, C], bf16, tag=f"wqk{g}")
            wqk_t.append(t)
        for g in (0, 1):
            nc.sync.dma_start(out=wqk_t[g][:], in_=wqkb_e[g])
        xt_t = []
        for k in range(KT):
            t = xts.tile([128, NL], bf16, tag=f"xt{k}")
            nc.sync.dma_start(out=t[:], in_=xt_e[k * 128:(k + 1) * 128, :])
            xt_t.append(t)
        bias_sb = const.tile([128, NBLK * 2 * WW], bf16, tag="bias")
        nc.sync.dma_start(out=bias_sb[:], in_=bias_e[:])
        wv_t = []
        for k in range(KT):
            t = wv_pool.tile([128, C], bf16, tag=f"wv{k}")
            nc.sync.dma_start(out=t[:], in_=wvt_e[k * 128:(k + 1) * 128, :])
            wv_t.append(t)
        for g in range(2, H):
            nc.sync.dma_start(out=wqk_t[g][:], in_=wqkb_e[g])
        wpt_t = []
        for k in range(KT):
            t = wpt_pool.tile([128, C], bf16, tag=f"wpt{k}")
            nc.sync.dma_start(out=t[:], in_=wpt_e[k])
            wpt_t.append(t)
        bpb_sb = const.tile([128, C], bf16, tag="bpb")
        nc.sync.dma_start(out=bpb_sb[:], in_=bpb_e[:])

        ident = const.tile([128, 128], bf16, tag="ident")
        make_identity(nc, ident[:])

        # persistent activation slabs; kxa = [k_even | 0], kxb = [0 | k_odd]
        # so score matmuls contract the full 128 partitions at base 0.
        q_sb, kxa_sb, kxb_sb = [], [], []
        for hp in range(8):
            tq = qk_pool.tile([128, NL], bf16, tag=f"q{hp}")
            q_sb.append(tq)
            ta = qk_pool.tile([128, NL], bf16, tag=f"kxa{hp}")
            nc.vector.memset(ta[64:128, :], 0.0)
            kxa_sb.append(ta)
            tb = qk_pool.tile([128, NL], bf16, tag=f"kxb{hp}")
            nc.vector.memset(tb[0:64, :], 0.0)
            kxb_sb.append(tb)
        v_sb = []
        for j in range(5):
            t = v_pool.tile([128, C], bf16, tag=f"v{j}")
            v_sb.append(t)
        ctxT = []
        for cb in range(8):
            t = ctx_pool.tile([128, NO], bf16, tag=f"ctx{cb}")
            ctxT.append(t)

        def emit_qk_with(g, ps0, ps1, copy):
            wt = wqk_t[g]
            for k in range(KT):
                nc.tensor.matmul(
                    ps0[:128, 0:264],
                    lhsT=wt[:, k * 128:(k + 1) * 128],
                    rhs=xt_t[k][:, 0:264],
                    start=(k == 0), stop=(k == KT - 1),
                )
                nc.tensor.matmul(
                    ps1[:128, 0:264],
                    lhsT=wt[:, k * 128:(k + 1) * 128],
                    rhs=xt_t[k][:, 264:528],
                    start=(k == 0), stop=(k == KT - 1),
                )
            copy(g, ps0, ps1)

        def qk_copies(g, ps0, ps1):
            if g % 2 == 0:
                nc.scalar.copy(out=q_sb[g // 2][:, 0:264], in_=ps0[:128, 0:264])
                nc.vector.tensor_copy(out=q_sb[g // 2][:, 264:528],
                                      in_=ps1[:128, 0:264])
            else:
                nc.scalar.copy(
                    out=kxa_sb[g // 2][0:64, 0:264], in_=ps0[0:64, 0:264])
                nc.vector.tensor_copy(
                    out=kxb_sb[g // 2][64:128, 0:264], in_=ps0[64:128, 0:264])
                nc.scalar.copy(
                    out=kxa_sb[g // 2][0:64, 264:528], in_=ps1[0:64, 0:264])
                nc.vector.tensor_copy(
                    out=kxb_sb[g // 2][64:128, 264:528], in_=ps1[64:128, 0:264])

        # ---- phase V (prefixed by qk0/qk1 warm-up while wv streams) ----
        with tc.tile_pool(name="psv", bufs=2, space="PSUM") as psv:
            for g in (0, 1):
                p0 = psv.tile([128, 512], f32, tag="psv0")
                p1 = psv.tile([128, 512], f32, tag="psv1")
                emit_qk_with(g, p0, p1, qk_copies)
            for jb in range(5):
                pj = 128 if jb < 4 else 16
                ps0 = psv.tile([128, 512], f32, tag="psv0")
                ps1 = psv.tile([128, 512], f32, tag="psv1")
                for k in range(KT):
                    nc.tensor.matmul(
                        ps0[:pj, :],
                        lhsT=xt_t[k][:, jb * 128:jb * 128 + pj],
                        rhs=wv_t[k][:, 0:512],
                        start=(k == 0), stop=(k == KT - 1),
                    )
                    nc.tensor.matmul(
                        ps1[:pj, :],
                        lhsT=xt_t[k][:, jb * 128:jb * 128 + pj],
                        rhs=wv_t[k][:, 512:1024],
                        start=(k == 0), stop=(k == KT - 1),
                    )
                nc.scalar.copy(out=v_sb[jb][:pj, 0:512], in_=ps0[:pj, :])
                nc.scalar.copy(out=v_sb[jb][:pj, 512:1024], in_=ps1[:pj, :])

        # ---- main: QK GEMM interleaved with attention ------------------
        with tc.tile_pool(name="psqk", bufs=2, space="PSUM") as psqk, \
             tc.tile_pool(name="pss", bufs=2, space="PSUM") as pss_pool, \
             tc.tile_pool(name="pst", bufs=1, space="PSUM") as pst_pool, \
             tc.tile_pool(name="psc", bufs=1, space="PSUM") as psc_pool:
            state = {}

            def emit_qk(g):
                ps0 = psqk.tile([128, 264], f32, tag="psqk0")
                ps1 = psqk.tile([128, 264], f32, tag="psqk1")
                emit_qk_with(g, ps0, ps1, qk_copies)

            def emit_scores(hp, blk):
                j0 = blk * 128
                q0 = HALO + blk * 128
                b0 = blk * 2 * WW
                ps = pss_pool.tile([128, 2 * WW], f32, tag="pss")
                nc.tensor.matmul(
                    ps[:, 0:WW], lhsT=ident[:, 0:128],
                    rhs=bias_sb[:, b0:b0 + WW], start=True, stop=False)
                nc.tensor.matmul(
                    ps[:, 0:WW], lhsT=q_sb[hp][:, q0:q0 + 128],
                    rhs=kxa_sb[hp][:, j0:j0 + WW], start=False, stop=True)
                nc.tensor.matmul(
                    ps[:, WW:2 * WW], lhsT=ident[:, 0:128],
                    rhs=bias_sb[:, b0 + WW:b0 + 2 * WW], start=True, stop=False)
                nc.tensor.matmul(
                    ps[:, WW:2 * WW], lhsT=q_sb[hp][:, q0:q0 + 128],
                    rhs=kxb_sb[hp][:, j0:j0 + WW], start=False, stop=True)
                ex = ex_pool.tile([128, 2 * WW], bf16, tag="ex")
                den = dn_pool.tile([128, 4], f32, tag="den")
                for hi in range(2):
                    nc.scalar.activation(
                        out=ex[:, hi * WW:(hi + 1) * WW],
                        in_=ps[:, hi * WW:(hi + 1) * WW],
                        func=EXP, accum_out=den[:, hi:hi + 1])
                nc.vector.reciprocal(out=den[:, 2:4], in_=den[:, 0:2])
                at = at_pool.tile([128, 2 * WW], bf16, tag="at")
                for hi in range(2):
                    nc.vector.tensor_scalar_mul(
                        out=at[:, hi * WW:(hi + 1) * WW],
                        in0=ex[:, hi * WW:(hi + 1) * WW],
                        scalar1=den[:, 2 + hi:3 + hi])
                state[(hp, blk)] = at

            def emit_attnout(hp):
                pc = psc_pool.tile([128, NO], f32, tag="psc")
                for blk in range(NBLK):
                    at = state.pop((hp, blk))
                    # pt: [0:128]=hA main^T, [128:256]=hB main^T,
                    #     [0:16, 256:384]=hA tail^T, [0:16, 384:512]=hB tail^T
                    pt = pst_pool.tile([128, 512], bf16, tag="pt")
                    nc.tensor.transpose(pt[:, 0:128], at[:, 0:128], ident[:])
                    nc.tensor.transpose(pt[:, 128:256], at[:, WW:WW + 128],
                                        ident[:])
                    nc.tensor.transpose(pt[0:16, 256:384], at[:, 128:WW],
                                        ident[:])
                    nc.tensor.transpose(pt[0:16, 384:512], at[:, WW + 128:2 * WW],
                                        ident[:])
                    atT = atT_pool.tile([128, 512], bf16, tag="atT")
                    nc.vector.tensor_copy(out=atT[:, 0:256], in_=pt[:, 0:256])
                    nc.vector.tensor_copy(out=atT[0:16, 256:512],
                                          in_=pt[0:16, 256:512])
                    for hi in range(2):
                        h = 2 * hp + hi
                        nc.tensor.matmul(
                            pc[hi * 64:(hi + 1) * 64, blk * 128:(blk + 1) * 128],
                            lhsT=v_sb[blk][:, h * 64:(h + 1) * 64],
                            rhs=atT[:, hi * 128:hi * 128 + 128],
                            start=True, stop=False,
                        )
                        nc.tensor.matmul(
                            pc[hi * 64:(hi + 1) * 64, blk * 128:(blk + 1) * 128],
                            lhsT=v_sb[blk + 1][0:16, h * 64:(h + 1) * 64],
                            rhs=atT[0:16, 256 + hi * 128:256 + hi * 128 + 128],
                            start=False, stop=True,
                        )
                nc.scalar.copy(out=ctxT[hp][:], in_=pc[:])

            for hp in range(8):
                emit_scores(hp, 0)
                emit_scores(hp, 1)
                if 2 * hp + 2 < H:
                    emit_qk(2 * hp + 2)
                emit_scores(hp, 2)
                emit_scores(hp, 3)
                if 2 * hp + 3 < H:
                    emit_qk(2 * hp + 3)
                if hp >= 1:
                    emit_attnout(hp - 1)
            emit_attnout(7)

        # ---- phase O: out_nat[n, o] (ctxT-stationary, k-inner) ---------
        with tc.tile_pool(name="pso", bufs=2, space="PSUM") as pso:
            for t in range(4):
                ps0 = pso.tile([128, 512], f32, tag="pso0")
                ps1 = pso.tile([128, 512], f32, tag="pso1")
                for k in range(KT):
                    nc.tensor.matmul(
                        ps0[:],
                        lhsT=ctxT[k][:, t * 128:(t + 1) * 128],
                        rhs=wpt_t[k][:, 0:512],
                        start=(k == 0), stop=(k == KT - 1),
                    )
                    nc.tensor.matmul(
                        ps1[:],
                        lhsT=ctxT[k][:, t * 128:(t + 1) * 128],
                        rhs=wpt_t[k][:, 512:1024],
                        start=(k == 0), stop=(k == KT - 1),
                    )
                for oh, psx in ((0, ps0), (1, ps1)):
                    ot = out_pool.tile([128, 512], f32, tag="ot")
                    nc.vector.tensor_add(
                        out=ot[:], in0=psx[:],
                        in1=bpb_sb[:, oh * 512:(oh + 1) * 512])
                    nc.sync.dma_start(
                        out=out_e[t * 128:(t + 1) * 128, oh * 512:(oh + 1) * 512],
                        in_=ot[:])

    from concourse import mybir as _mb
    import os as _os
    if _os.environ.get("KERNEL_NO_DEDUP"):
        n = 0
    else:
        n = _dedup_ldweights(nc, _mb)
    _CACHE["ldweights_removed"] = n
    nc.compile()
    return nc


def _get_nc():
    if "nc" not in _CACHE:
        _CACHE["nc"] = _build_nc()
    return _CACHE["nc"]


def _band_width(epoch):
    if epoch is None or epoch >= 50:
        return None
    if epoch < 20:
        return 6
    if epoch < 30:
        return 8
    if epoch < 40:
        return 10
    return 12


def _numpy_ref(x, Wqkv, Wproj, bproj, w):
    """Pure-numpy fallback for band widths this kernel wasn't compiled for."""
    b, n, c = x.shape
    d = c // H
    qkv = np.einsum("bnc,oc->bno", x, Wqkv)
    qkv = qkv.reshape(b, n, H, 3 * d).transpose(0, 2, 1, 3)
    q, k, v = np.split(qkv, 3, axis=-1)
    score = np.einsum("bhid,bhjd->bhij", q, k) * (d ** -0.5)
    if w is not None:
        idx = np.arange(n)
        mask = np.abs(idx[:, None] - idx[None, :]) <= w
        score = np.where(mask[None, None], score, np.float32(-1e9))
    score -= score.max(axis=-1, keepdims=True)
    e = np.exp(score)
    attn = e / e.sum(axis=-1, keepdims=True)
    ctxv = np.einsum("bhij,bhjd->bhid", attn, v)
    ctxv = ctxv.transpose(0, 2, 1, 3).reshape(b, n, c)
    return (np.einsum("bnc,oc->bno", ctxv, Wproj) + bproj).astype(np.float32)


def _prep_in_maps(x, Wqkv, Wproj, bproj):
    import ml_dtypes
    bf = ml_dtypes.bfloat16
    x = np.ascontiguousarray(np.asarray(x, dtype=np.float32))
    Wqkv = np.asarray(Wqkv, dtype=np.float32)
    Wproj = np.asarray(Wproj, dtype=np.float32)
    bproj = np.asarray(bproj, dtype=np.float32)

    # qk weight output-blocks g: even g -> [q_{2hp} | q_{2hp+1}] (prescaled),
    # odd g -> [k_{2hp} | k_{2hp+1}]
    wsplit = Wqkv.reshape(H, 3, D, C)
    wq = wsplit[:, 0] * np.float32(SCALE)                      # [H, D, C]
    wk = wsplit[:, 1]                                          # [H, D, C]
    wv = wsplit[:, 2]                                          # [H, D, C]
    wg = np.empty((H, 128, C), dtype=np.float32)
    wg[0::2] = wq.reshape(8, 128, C)
    wg[1::2] = wk.reshape(8, 128, C)
    # wqkb[g, p, k*128+m] = wg[g, m, k*128+p]: per-g contiguous [128, C]
    # slabs whose col-block k is the k-th contraction tile's lhsT
    wqkb = np.ascontiguousarray(
        wg.transpose(0, 2, 1).reshape(H, KT, 128, 128).transpose(0, 2, 1, 3)
        .reshape(H, 128, C)).astype(bf)
    wvt = np.ascontiguousarray(wv.reshape(H * D, C).T).astype(bf)  # [C, C]
    wpt = np.ascontiguousarray(Wproj.T.reshape(KT, 128, C)).astype(bf)
    bpb = np.ascontiguousarray(
        np.broadcast_to(bproj[None, :], (128, C))).astype(bf)

    # additive score bias per sequence-half s: 0 where in-band and the k
    # column is a real token, else -1e9.  Layout [128, blk*(2*WW)] with the
    # per-block [128, WW] pattern duplicated for the two heads of a pair.
    r = np.arange(128)[:, None]
    jj = np.arange(WW)[None, :]
    band = (jj >= r) & (jj <= r + 2 * HALO)                    # [128, WW]
    biases = []
    for s in (0, 1):
        m = np.full((128, NBLK * 2 * WW), NEG, dtype=np.float32)
        for blk in range(NBLK):
            mloc = blk * 128 + jj                              # local k index
            valid = (mloc >= HALO) if s == 0 else (mloc < NO + HALO)
            bb = np.where(band & valid, 0.0, NEG).astype(np.float32)
            m[:, blk * 2 * WW:blk * 2 * WW + WW] = bb
            m[:, blk * 2 * WW + WW:(blk + 1) * 2 * WW] = bb
        biases.append(m.astype(bf))

    in_maps = []
    for core in range(8):
        b, s = core // 2, core % 2
        xloc = np.zeros((NL, C), dtype=np.float32)
        g0 = s * NO - HALO
        lo, hi = max(0, g0), min(N, g0 + NL)
        xloc[lo - g0:hi - g0] = x[b, lo:hi]
        in_maps.append({
            "xt": np.ascontiguousarray(xloc.T).astype(bf),
            "wqkb": wqkb, "wvt": wvt, "wpt": wpt, "bpb": bpb,
            "bias": biases[s],
        })
    return in_maps


def kernel(x, Wqkv, Wproj, bproj, epoch):
    ep = None if epoch is None else int(np.asarray(epoch))
    w = _band_width(ep)
    if w != HALO:
        return _numpy_ref(np.asarray(x, np.float32), np.asarray(Wqkv, np.float32),
                          np.asarray(Wproj, np.float32),
                          np.asarray(bproj, np.float32), w)

    from concourse.bass_utils import run_bass_kernel_spmd

    nc = _get_nc()
    in_maps = _prep_in_maps(x, Wqkv, Wproj, bproj)
    res = run_bass_kernel_spmd(nc, in_maps, core_ids=list(range(8)))
    _CACHE["last_results"] = res

    out = np.empty((B, N, C), dtype=np.float32)
    for core in range(8):
        b, s = core // 2, core % 2
        out[b, s * NO:(s + 1) * NO, :] = res.results[core]["out"]
    return out
